# revision 37
# baseline (speedup 1.0000x reference)
"""Trainium2 Bass kernel for nn_DressedQuantumNet.

Math reformulation (exact, up to float rounding):
  pre_out = x @ pre_w.T + pre_b                  # [B,4]
  theta_w = (pi/4)*tanh(pre_out_w) + pi/4        # in (0, pi/2)
  v_w     = [cos theta_w, sin theta_w]           # per-qubit state (positive)
  psi     = v_0 (x) v_1 (x) v_2 (x) v_3          # [B,16] product state
  phi     = M @ psi        # M = fixed 16x16 matrix of the CNOT/RY circuit
  out     = (phi*phi)^T P + post_b  # P[i,c] = sum_w post_w[c,w] * z_w(i)

Device strategy (pure data parallel over 8 cores, 8192 samples each):
  - x is downcast to bf16 on host (halves HBM traffic; fp32 accumulation in
    PSUM keeps the matmul accurate).
  - x tiles are loaded transposed via the DMA xbar (dma transpose), so the
    contraction dim (D=512, in 4 chunks of 128) lands on SBUF partitions.
  - pre-matmul: lhsT = xT chunk [128d, 128b], rhs = pre_w^T chunk [128d, 4]
    accumulated over the 4 chunks into PSUM [128b, 4].
  - bias + PSUM->SBUF handled by one vector add with a broadcast bias AP.
  - angles/trig on ScalarE (Tanh + 2x Sin with scale/bias folding cos).
  - psi built with 3 broadcast-AP vector multiplies.
  - quantum circuit: PE transpose of psi -> [16 comps x 8 tiles, 128 samples],
    then two block-diagonal matmuls (M and P) on the tensor engine.
"""

import os
import sys

for _p in ("/opt/trn_rl_repo",):
    if os.path.isdir(_p) and _p not in sys.path:
        sys.path.insert(0, _p)

import math
import numpy as np
import ml_dtypes
from contextlib import ExitStack

import concourse.bass as bass
import concourse.bacc as bacc
import concourse.mybir as mybir
from concourse.tile import TileContext, add_dep_helper
from concourse.bass_utils import run_bass_kernel_spmd

F32 = mybir.dt.float32
BF16 = mybir.dt.bfloat16
AF = mybir.ActivationFunctionType
PI4 = math.pi / 4.0

N_CORES = 8
B_FULL, D, C = 65536, 512, 10
B = B_FULL // N_CORES          # 8192 samples per core
N_QUBITS, Q_DEPTH = 4, 6
TILES = B // 128               # 64 sample tiles of 128
GROUPS = 16                    # phase-1 groups of 512 samples (4 tiles)
CHUNKS = 4                     # phase-2 chunks of 2048 samples (16 tiles)
G_PER_C = GROUPS // CHUNKS


# ---------------------------------------------------------------- host math
def _apply_1q(state, gate, wire):
    state = np.moveaxis(state, wire, 0)
    state = np.tensordot(gate, state, axes=((1,), (0,)))
    return np.moveaxis(state, 0, wire)


def _apply_cnot(state, ctrl, tgt):
    state = np.moveaxis(state, (ctrl, tgt), (0, 1))
    state = np.stack([state[0], state[1][::-1]], axis=0)
    return np.moveaxis(state, (0, 1), (ctrl, tgt))


def _ry(theta):
    c, s = np.cos(theta * 0.5), np.sin(theta * 0.5)
    return np.array([[c, -s], [s, c]])


def _build_M(q_params: np.ndarray) -> np.ndarray:
    """16x16 matrix of the fixed part of the circuit (after the per-sample
    RY layer): 6 repetitions of [CNOT(0,1), CNOT(2,3), CNOT(1,2), RY layer]."""
    qw = np.asarray(q_params, np.float64).reshape(Q_DEPTH, N_QUBITS)
    M = np.zeros((16, 16), np.float64)
    for i in range(16):
        state = np.zeros(16, np.float64)
        state[i] = 1.0
        state = state.reshape((2,) * N_QUBITS)
        for k in range(Q_DEPTH):
            for a in range(0, N_QUBITS - 1, 2):
                state = _apply_cnot(state, a, a + 1)
            for a in range(1, N_QUBITS - 1, 2):
                state = _apply_cnot(state, a, a + 1)
            for w in range(N_QUBITS):
                state = _apply_1q(state, _ry(qw[k, w]), w)
        M[:, i] = state.reshape(16)
    return M


def _build_P(post_w: np.ndarray) -> np.ndarray:
    """P[i, c] = sum_w post_w[c, w] * z_w(i), where z_w(i) flips sign with
    bit (3-w) of the state index i (axis 0 of the state = qubit 0)."""
    post_w = np.asarray(post_w, np.float64)
    i = np.arange(16)
    z = np.stack([1.0 - 2.0 * ((i >> (3 - w)) & 1) for w in range(N_QUBITS)], 1)
    return z @ post_w.T  # [16, 10]


# ---------------------------------------------------------------- bass build
def build_nc() -> bass.Bass:
    # Bacc (not raw Bass): its finalize() runs generate_event_semaphores,
    # which splits multi-semaphore waits to satisfy the TRN2 one-wait-per-
    # instruction ISA limit.
    nc = bacc.Bacc(None)
    x = nc.dram_tensor("x", [B, D], BF16, kind="ExternalInput")
    pre_wt = nc.dram_tensor("pre_wt", [128, 16], BF16, kind="ExternalInput")
    # one row: cols 0..127 = ones (rank-1 bias matmul lhsT), 128..131 = pre_b
    bias_pack = nc.dram_tensor("bias_pack", [1, 132], BF16,
                               kind="ExternalInput")
    mbd = nc.dram_tensor("mbd", [128, 128], F32, kind="ExternalInput")
    pbd = nc.dram_tensor("pbd", [128, 80], F32, kind="ExternalInput")
    post_b80 = nc.dram_tensor("post_b80", [80, 1], F32, kind="ExternalInput")
    trigb = nc.dram_tensor("trigb", [128, 2], F32, kind="ExternalInput")
    ident = nc.dram_tensor("ident", [128, 128], F32, kind="ExternalInput")
    out = nc.dram_tensor("out", [B, C], F32, kind="ExternalOutput")

    with ExitStack() as ctx:
        tc = ctx.enter_context(TileContext(nc))
        consts = ctx.enter_context(tc.tile_pool(name="consts", bufs=1))
        # all 64 xT tiles stay resident (8 MB) — avoids WAR waits on the
        # transpose DMAs (DmaTransposeAnt supports very few sync waits)
        xt_pool = ctx.enter_context(tc.tile_pool(name="xt", bufs=GROUPS))
        work = ctx.enter_context(tc.tile_pool(name="work", bufs=2))
        ps_po = ctx.enter_context(tc.tile_pool(name="ps_po", space="PSUM", bufs=2))
        ps2 = ctx.enter_context(tc.tile_pool(name="ps2", space="PSUM", bufs=4))

        pre_wt_sb = consts.tile([128, 16], BF16)
        nc.gpsimd.dma_start(pre_wt_sb, pre_wt[:, :])
        pack_sb = consts.tile([1, 132], BF16)
        nc.gpsimd.dma_start(pack_sb, bias_pack[:, :])
        ones_sb = pack_sb[0:1, 0:128]
        pre_b_sb = pack_sb[0:1, 128:132]
        mbd_sb = consts.tile([128, 128], F32)
        nc.gpsimd.dma_start(mbd_sb, mbd[:, :])
        pbd_sb = consts.tile([128, 80], F32)
        nc.gpsimd.dma_start(pbd_sb, pbd[:, :])
        pb80_sb = consts.tile([80, 1], F32)
        nc.gpsimd.dma_start(pb80_sb, post_b80[:, :])
        trigb_sb = consts.tile([128, 2], F32)
        nc.gpsimd.dma_start(trigb_sb, trigb[:, :])
        id_sb = consts.tile([128, 128], F32)
        last_const = nc.gpsimd.dma_start(id_sb, ident[:, :])

        pre_out_sb = consts.tile([128, 4 * TILES], F32)  # [128, 256]
        out_sb = consts.tile([128, 80 * 2 * CHUNKS], F32)  # [128, 640]

        # Sacrificial reads so each engine "observes" the const-load DMA
        # semaphore lanes up front.  Matmult/transpose ISA structs can encode
        # only ONE sync wait, so a later matmul must never need both a data
        # wait and an unobserved const-lane wait.
        dummy_ps = ps2.tile([128, 128], F32, name="dummy_ps", tag="dummy",
                            bufs=1)
        dummy_sb = consts.tile([128, 2], F32)
        for cap in (id_sb[:, :], mbd_sb[:, :], pbd_sb[:, :]):
            nc.tensor.matmul(dummy_ps[:cap.shape[1], :cap.shape[1]],
                             lhsT=cap, rhs=cap,
                             start=True, stop=True, skip_group_check=True)
        nc.tensor.matmul(dummy_ps[:16, :16], lhsT=pre_wt_sb[:, :],
                         rhs=pre_wt_sb[:, :], start=True, stop=True,
                         skip_group_check=True)
        nc.tensor.matmul(dummy_ps[:128, :4], lhsT=ones_sb,
                         rhs=pre_b_sb, start=True, stop=True,
                         skip_group_check=True)
        nc.scalar.copy(dummy_sb[:80, 0:1], pb80_sb[:, :])
        nc.scalar.copy(dummy_sb[:, 1:2], trigb_sb[:, 0:1])

        for cki in range(CHUNKS):
            # ---- phase 1: pre-net for this chunk's 4 groups of 512 samples
            for gi in range(G_PER_C):
                g = cki * G_PER_C + gi
                xts = []
                for k in range(4):
                    xt = xt_pool.tile([128, 512], BF16, name=f"xt{k}",
                                      tag=f"xt{k}")
                    xpose = nc.sync.dma_start(
                        xt, x[512 * g:512 * (g + 1), 128 * k:128 * (k + 1)],
                        transpose=True)
                    # keep all plain copies strictly before all xbar
                    # transposes: the DmaTransposeAnt ISA struct can encode
                    # only one sync wait, so each copy<->transpose mode
                    # transition must not land on a lane-reusing transpose
                    add_dep_helper(last_const.ins, xpose.ins, sync=False,
                                   reason="consts before xbar transposes")
                    xts.append(xt)
                po = ps_po.tile([128, 16], F32, name="po", tag="po")
                # memset absorbs the PSUM-slot WAR + bank-guard waits (the
                # Matmult ISA struct can encode only one sync wait); the
                # matmuls then accumulate onto zeros with start=False
                nc.vector.memset(po[:, :], 0.0)
                for s in range(4):
                    # bias: pre_out += ones^T @ pre_b  (rank-1, K=1)
                    nc.tensor.matmul(
                        po[:, 4 * s:4 * (s + 1)],
                        lhsT=ones_sb, rhs=pre_b_sb,
                        start=False, stop=False, skip_group_check=True)
                    for k in range(4):
                        nc.tensor.matmul(
                            po[:, 4 * s:4 * (s + 1)],
                            lhsT=xts[k][:, 128 * s:128 * (s + 1)],
                            rhs=pre_wt_sb[:, 4 * k:4 * (k + 1)],
                            start=False, stop=(k == 3),
                            skip_group_check=True)
                nc.vector.tensor_copy(
                    pre_out_sb[:, 16 * g:16 * (g + 1)], po[:, :])

            # ---- phase 2: trig + quantum net for this chunk (16 tiles)
            th = work.tile([128, 64], F32, name="th", tag="th")
            nc.scalar.activation(
                th, pre_out_sb[:, 64 * cki:64 * (cki + 1)], AF.Tanh)
            cs = work.tile([128, 128], F32, name="cs", tag="cs")
            cs4 = cs[:, :].rearrange("p (t w x) -> p t w x", w=4, x=2)
            th3 = th[:, :].rearrange("p (t w) -> p t w", w=4)
            # cos(theta) = sin(pi/4*tanh + 3pi/4); sin(theta) = sin(.. + pi/4)
            nc.scalar.activation(cs4[:, :, :, 0], th3, AF.Sin,
                                 bias=trigb_sb[:, 0:1], scale=PI4)
            nc.scalar.activation(cs4[:, :, :, 1], th3, AF.Sin,
                                 bias=trigb_sb[:, 1:2], scale=PI4)
            cs8 = cs[:, :].rearrange("p (t w) -> p t w", w=8)
            v01 = work.tile([128, 64], F32, name="v01", tag="v01")
            v23 = work.tile([128, 64], F32, name="v23", tag="v23")
            nc.vector.tensor_tensor(
                out=v01[:, :].rearrange("p (t a b) -> p t a b", a=2, b=2),
                in0=cs8[:, :, 0:2].unsqueeze(3).broadcast_to((128, 16, 2, 2)),
                in1=cs8[:, :, 2:4].unsqueeze(2).broadcast_to((128, 16, 2, 2)),
                op=mybir.AluOpType.mult)
            nc.vector.tensor_tensor(
                out=v23[:, :].rearrange("p (t a b) -> p t a b", a=2, b=2),
                in0=cs8[:, :, 4:6].unsqueeze(3).broadcast_to((128, 16, 2, 2)),
                in1=cs8[:, :, 6:8].unsqueeze(2).broadcast_to((128, 16, 2, 2)),
                op=mybir.AluOpType.mult)
            psi = work.tile([128, 256], F32, name="psi", tag="psi")
            nc.vector.tensor_tensor(
                out=psi[:, :].rearrange("p (t a b) -> p t a b", a=4, b=4),
                in0=v01[:, :].rearrange("p (t i) -> p t i", i=4)
                    .unsqueeze(3).broadcast_to((128, 16, 4, 4)),
                in1=v23[:, :].rearrange("p (t i) -> p t i", i=4)
                    .unsqueeze(2).broadcast_to((128, 16, 4, 4)),
                op=mybir.AluOpType.mult)

            for h in range(2):
                # every PE->PSUM write is preceded by a DVE memset of the
                # slot: it soaks up the slot-WAR + bank-guard waits so the
                # matmul/transpose itself needs only its single data wait
                psiT_ps = ps2.tile([128, 128], F32, name="psiT_ps", tag="p2")
                nc.vector.memset(psiT_ps[:, :], 0.0)
                nc.tensor.transpose(
                    psiT_ps, psi[:, 128 * h:128 * (h + 1)], id_sb[:, :])
                psiT = work.tile([128, 128], F32, name="psiT", tag="psiT")
                nc.vector.tensor_copy(psiT, psiT_ps)
                phiT_ps = ps2.tile([128, 128], F32, name="phiT_ps", tag="p2")
                nc.vector.memset(phiT_ps[:, :], 0.0)
                nc.tensor.matmul(phiT_ps, lhsT=mbd_sb[:, :], rhs=psiT,
                                 start=True, stop=True)
                phi2 = work.tile([128, 128], F32, name="phi2", tag="phi2")
                nc.scalar.activation(phi2, phiT_ps, AF.Square)
                o10_ps = ps2.tile([80, 128], F32, name="o10_ps", tag="p2")
                nc.scalar.memzero(o10_ps[:, :])
                nc.tensor.matmul(o10_ps, lhsT=pbd_sb[:, :], rhs=phi2,
                                 start=True, stop=True)
                o10 = work.tile([80, 128], F32, name="o10", tag="o10")
                nc.scalar.activation(o10, o10_ps, AF.Identity,
                                     bias=pb80_sb[:, :])
                ob_ps = ps2.tile([128, 80], F32, name="ob_ps", tag="p2")
                nc.scalar.memzero(ob_ps[:, :])
                nc.tensor.transpose(ob_ps, o10[:, :], id_sb[:80, :80])
                hh = 2 * cki + h
                nc.vector.tensor_copy(out_sb[:, 80 * hh:80 * (hh + 1)], ob_ps)

        # single store at the very end — no copy<->transpose mode transition
        # lands between xbar transposes (see comment at the transpose DMAs)
        nc.scalar.dma_start(
            out[:, :].rearrange("(h t p) c -> p h t c", p=128, t=8),
            out_sb[:, :].rearrange("p (h t c) -> p h t c", t=8, c=10))

    nc.finalize()  # bacc: register alloc + event-semaphore wait splitting
    return nc


_NC_CACHE: dict = {}


def _get_nc() -> bass.Bass:
    if "nc" not in _NC_CACHE:
        _NC_CACHE["nc"] = build_nc()
    return _NC_CACHE["nc"]


def make_in_maps(inputs: dict) -> list:
    x = np.asarray(inputs["input_features"], np.float32)
    pre_w = np.asarray(inputs["pre_w"], np.float32)
    pre_b = np.asarray(inputs["pre_b"], np.float32)
    q_params = np.asarray(inputs["q_params"], np.float32)
    post_w = np.asarray(inputs["post_w"], np.float32)
    post_b = np.asarray(inputs["post_b"], np.float32)

    M = _build_M(q_params)
    P = _build_P(post_w)
    mbd = np.zeros((128, 128), np.float32)
    pbd = np.zeros((128, 80), np.float32)
    for t in range(8):
        mbd[16 * t:16 * (t + 1), 16 * t:16 * (t + 1)] = M.T
        pbd[16 * t:16 * (t + 1), 10 * t:10 * (t + 1)] = P
    # pre_wt_sb[p, 4k+f] = pre_w[f, 128k+p]
    pre_wt = np.ascontiguousarray(
        pre_w.T.reshape(4, 128, 4).transpose(1, 0, 2).reshape(128, 16)
    ).astype(ml_dtypes.bfloat16)
    bias_pack = np.zeros((1, 132), dtype=ml_dtypes.bfloat16)
    bias_pack[0, :128] = 1.0
    bias_pack[0, 128:] = pre_b.astype(ml_dtypes.bfloat16)
    post_b80 = np.ascontiguousarray(np.tile(post_b, 8).reshape(80, 1))
    trigb = np.ascontiguousarray(np.broadcast_to(
        np.array([3.0 * PI4, PI4], np.float32), (128, 2)))
    ident = np.eye(128, dtype=np.float32)

    xb = x.astype(ml_dtypes.bfloat16)
    consts = dict(pre_wt=pre_wt, bias_pack=bias_pack, mbd=mbd, pbd=pbd,
                  post_b80=post_b80, trigb=trigb, ident=ident)
    return [dict(x=xb[B * i:B * (i + 1)], **consts) for i in range(N_CORES)]


def run_on_device(inputs: dict, **kwargs):
    """Returns (full_output, BassKernelResults)."""
    nc = _get_nc()
    in_maps = make_in_maps(inputs)
    res = run_bass_kernel_spmd(nc, in_maps, core_ids=list(range(N_CORES)),
                               **kwargs)
    full = np.concatenate([res.results[i]["out"] for i in range(N_CORES)], 0)
    return np.ascontiguousarray(full, dtype=np.float32), res


def kernel(**inputs) -> np.ndarray:
    out, _ = run_on_device(inputs)
    return out


# revision 40
# speedup vs baseline: 1.7105x; 1.7105x over previous
"""Trainium2 Bass kernel for nn_DressedQuantumNet.

Math reformulation (exact, up to float rounding):
  pre_out = x @ pre_w.T + pre_b                  # [B,4]
  theta_w = (pi/4)*tanh(pre_out_w) + pi/4        # in (0, pi/2)
  v_w     = [cos theta_w, sin theta_w]           # per-qubit state (positive)
  psi     = v_0 (x) v_1 (x) v_2 (x) v_3          # [B,16] product state
  phi     = M @ psi        # M = fixed 16x16 matrix of the CNOT/RY circuit
  out     = (phi*phi)^T P + post_b  # P[i,c] = sum_w post_w[c,w] * z_w(i)

Device strategy (pure data parallel over 8 cores, 8192 samples each):
  - x is downcast to bf16 on host (halves HBM traffic; fp32 accumulation in
    PSUM keeps the matmul accurate).
  - x tiles are loaded transposed via the DMA xbar (dma transpose), so the
    contraction dim (D=512, in 4 chunks of 128) lands on SBUF partitions.
  - pre-matmul: lhsT = xT chunk [128d, 128b], rhs = pre_w^T chunk [128d, 4]
    accumulated over the 4 chunks into PSUM [128b, 4].
  - bias + PSUM->SBUF handled by one vector add with a broadcast bias AP.
  - angles/trig on ScalarE (Tanh + 2x Sin with scale/bias folding cos).
  - psi built with 3 broadcast-AP vector multiplies.
  - quantum circuit: PE transpose of psi -> [16 comps x 8 tiles, 128 samples],
    then two block-diagonal matmuls (M and P) on the tensor engine.
"""

import os
import sys

for _p in ("/opt/trn_rl_repo",):
    if os.path.isdir(_p) and _p not in sys.path:
        sys.path.insert(0, _p)

import math
import numpy as np
import ml_dtypes
from contextlib import ExitStack

import concourse.bass as bass
import concourse.bacc as bacc
import concourse.mybir as mybir
from concourse.tile import TileContext, add_dep_helper
from concourse.bass_utils import run_bass_kernel_spmd

F32 = mybir.dt.float32
BF16 = mybir.dt.bfloat16
AF = mybir.ActivationFunctionType
PI4 = math.pi / 4.0

N_CORES = 8
B_FULL, D, C = 65536, 512, 10
B = B_FULL // N_CORES          # 8192 samples per core
N_QUBITS, Q_DEPTH = 4, 6
TILES = B // 128               # 64 sample tiles of 128
GROUPS = 16                    # phase-1 groups of 512 samples (4 tiles)
CHUNKS = 4                     # phase-2 chunks of 2048 samples (16 tiles)
G_PER_C = GROUPS // CHUNKS


# ---------------------------------------------------------------- host math
def _apply_1q(state, gate, wire):
    state = np.moveaxis(state, wire, 0)
    state = np.tensordot(gate, state, axes=((1,), (0,)))
    return np.moveaxis(state, 0, wire)


def _apply_cnot(state, ctrl, tgt):
    state = np.moveaxis(state, (ctrl, tgt), (0, 1))
    state = np.stack([state[0], state[1][::-1]], axis=0)
    return np.moveaxis(state, (0, 1), (ctrl, tgt))


def _ry(theta):
    c, s = np.cos(theta * 0.5), np.sin(theta * 0.5)
    return np.array([[c, -s], [s, c]])


def _build_M(q_params: np.ndarray) -> np.ndarray:
    """16x16 matrix of the fixed part of the circuit (after the per-sample
    RY layer): 6 repetitions of [CNOT(0,1), CNOT(2,3), CNOT(1,2), RY layer]."""
    qw = np.asarray(q_params, np.float64).reshape(Q_DEPTH, N_QUBITS)
    M = np.zeros((16, 16), np.float64)
    for i in range(16):
        state = np.zeros(16, np.float64)
        state[i] = 1.0
        state = state.reshape((2,) * N_QUBITS)
        for k in range(Q_DEPTH):
            for a in range(0, N_QUBITS - 1, 2):
                state = _apply_cnot(state, a, a + 1)
            for a in range(1, N_QUBITS - 1, 2):
                state = _apply_cnot(state, a, a + 1)
            for w in range(N_QUBITS):
                state = _apply_1q(state, _ry(qw[k, w]), w)
        M[:, i] = state.reshape(16)
    return M


def _build_P(post_w: np.ndarray) -> np.ndarray:
    """P[i, c] = sum_w post_w[c, w] * z_w(i), where z_w(i) flips sign with
    bit (3-w) of the state index i (axis 0 of the state = qubit 0)."""
    post_w = np.asarray(post_w, np.float64)
    i = np.arange(16)
    z = np.stack([1.0 - 2.0 * ((i >> (3 - w)) & 1) for w in range(N_QUBITS)], 1)
    return z @ post_w.T  # [16, 10]


# ---------------------------------------------------------------- bass build
def build_nc() -> bass.Bass:
    # Bacc (not raw Bass): its finalize() runs generate_event_semaphores,
    # which splits multi-semaphore waits to satisfy the TRN2 one-wait-per-
    # instruction ISA limit.
    nc = bacc.Bacc(None)
    x = nc.dram_tensor("x", [B, D], BF16, kind="ExternalInput")
    pre_wt = nc.dram_tensor("pre_wt", [128, 16], BF16, kind="ExternalInput")
    # one row: cols 0..127 = ones (rank-1 bias matmul lhsT), 128..131 = pre_b
    bias_pack = nc.dram_tensor("bias_pack", [1, 132], BF16,
                               kind="ExternalInput")
    mbd = nc.dram_tensor("mbd", [128, 128], F32, kind="ExternalInput")
    pbd = nc.dram_tensor("pbd", [128, 80], F32, kind="ExternalInput")
    post_b80 = nc.dram_tensor("post_b80", [80, 1], F32, kind="ExternalInput")
    trigb = nc.dram_tensor("trigb", [128, 2], F32, kind="ExternalInput")
    ident = nc.dram_tensor("ident", [128, 128], F32, kind="ExternalInput")
    # transposed on device: out[tile, class, sample-in-tile]; host flips back
    out = nc.dram_tensor("out", [TILES, C, 128], F32, kind="ExternalOutput")

    with ExitStack() as ctx:
        tc = ctx.enter_context(TileContext(nc))
        consts = ctx.enter_context(tc.tile_pool(name="consts", bufs=1))
        # all 64 xT tiles stay resident (8 MB) — avoids WAR waits on the
        # transpose DMAs (DmaTransposeAnt supports very few sync waits)
        xt_pool = ctx.enter_context(tc.tile_pool(name="xt", bufs=GROUPS))
        work = ctx.enter_context(tc.tile_pool(name="work", bufs=2))
        ps_po = ctx.enter_context(tc.tile_pool(name="ps_po", space="PSUM", bufs=2))
        ps2 = ctx.enter_context(tc.tile_pool(name="ps2", space="PSUM", bufs=4))

        pre_wt_sb = consts.tile([128, 16], BF16)
        nc.gpsimd.dma_start(pre_wt_sb, pre_wt[:, :])
        pack_sb = consts.tile([1, 132], BF16)
        nc.gpsimd.dma_start(pack_sb, bias_pack[:, :])
        ones_sb = pack_sb[0:1, 0:128]
        pre_b_sb = pack_sb[0:1, 128:132]
        mbd_sb = consts.tile([128, 128], F32)
        nc.gpsimd.dma_start(mbd_sb, mbd[:, :])
        pbd_sb = consts.tile([128, 80], F32)
        nc.gpsimd.dma_start(pbd_sb, pbd[:, :])
        pb80_sb = consts.tile([80, 1], F32)
        nc.gpsimd.dma_start(pb80_sb, post_b80[:, :])
        trigb_sb = consts.tile([128, 2], F32)
        nc.gpsimd.dma_start(trigb_sb, trigb[:, :])
        id_sb = consts.tile([128, 128], F32)
        last_const = nc.gpsimd.dma_start(id_sb, ident[:, :])

        pre_out_sb = consts.tile([128, 4 * TILES], F32)  # [128, 256]
        out2_sb = consts.tile([80, 128 * 2 * CHUNKS], F32)  # [80, 1024]

        for cki in range(CHUNKS):
            # ---- phase 1: pre-net for this chunk's 4 groups of 512 samples
            for gi in range(G_PER_C):
                g = cki * G_PER_C + gi
                # one xbar transpose per 512-sample group with a fully
                # contiguous 512KB DRAM source: out[p, k, b] = x[b, 128k+p]
                xt = xt_pool.tile([128, 4 * 512], BF16, name="xt", tag="xt")
                xpose = nc.sync.dma_start(
                    xt[:, :].rearrange("p (k b) -> p k b", k=4),
                    x[512 * g:512 * (g + 1), :],
                    transpose=True)
                # keep all plain copies scheduled before all xbar transposes
                # (every copy<->transpose transition serializes the DMA ring)
                add_dep_helper(last_const.ins, xpose.ins, sync=False,
                               reason="consts before xbar transposes")
                po = ps_po.tile([128, 16], F32, name="po", tag="po")
                for s in range(4):
                    # bias: pre_out += ones^T @ pre_b  (rank-1, K=1)
                    nc.tensor.matmul(
                        po[:, 4 * s:4 * (s + 1)],
                        lhsT=ones_sb, rhs=pre_b_sb,
                        start=True, stop=False)
                    for k in range(4):
                        nc.tensor.matmul(
                            po[:, 4 * s:4 * (s + 1)],
                            lhsT=xt[:, 512 * k + 128 * s:512 * k + 128 * (s + 1)],
                            rhs=pre_wt_sb[:, 4 * k:4 * (k + 1)],
                            start=False, stop=(k == 3))
                nc.vector.tensor_copy(
                    pre_out_sb[:, 16 * g:16 * (g + 1)], po[:, :])

            # ---- phase 2: trig + quantum net for this chunk (16 tiles)
            th = work.tile([128, 64], F32, name="th", tag="th")
            nc.scalar.activation(
                th, pre_out_sb[:, 64 * cki:64 * (cki + 1)], AF.Tanh)
            cs = work.tile([128, 128], F32, name="cs", tag="cs")
            cs4 = cs[:, :].rearrange("p (t w x) -> p t w x", w=4, x=2)
            th3 = th[:, :].rearrange("p (t w) -> p t w", w=4)
            # cos(theta) = sin(pi/4*tanh + 3pi/4); sin(theta) = sin(.. + pi/4)
            nc.scalar.activation(cs4[:, :, :, 0], th3, AF.Sin,
                                 bias=trigb_sb[:, 0:1], scale=PI4)
            nc.scalar.activation(cs4[:, :, :, 1], th3, AF.Sin,
                                 bias=trigb_sb[:, 1:2], scale=PI4)
            cs8 = cs[:, :].rearrange("p (t w) -> p t w", w=8)
            v01 = work.tile([128, 64], F32, name="v01", tag="v01")
            v23 = work.tile([128, 64], F32, name="v23", tag="v23")
            nc.vector.tensor_tensor(
                out=v01[:, :].rearrange("p (t a b) -> p t a b", a=2, b=2),
                in0=cs8[:, :, 0:2].unsqueeze(3).broadcast_to((128, 16, 2, 2)),
                in1=cs8[:, :, 2:4].unsqueeze(2).broadcast_to((128, 16, 2, 2)),
                op=mybir.AluOpType.mult)
            nc.vector.tensor_tensor(
                out=v23[:, :].rearrange("p (t a b) -> p t a b", a=2, b=2),
                in0=cs8[:, :, 4:6].unsqueeze(3).broadcast_to((128, 16, 2, 2)),
                in1=cs8[:, :, 6:8].unsqueeze(2).broadcast_to((128, 16, 2, 2)),
                op=mybir.AluOpType.mult)
            psi = work.tile([128, 256], F32, name="psi", tag="psi")
            nc.vector.tensor_tensor(
                out=psi[:, :].rearrange("p (t a b) -> p t a b", a=4, b=4),
                in0=v01[:, :].rearrange("p (t i) -> p t i", i=4)
                    .unsqueeze(3).broadcast_to((128, 16, 4, 4)),
                in1=v23[:, :].rearrange("p (t i) -> p t i", i=4)
                    .unsqueeze(2).broadcast_to((128, 16, 4, 4)),
                op=mybir.AluOpType.mult)

            for h in range(2):
                psiT_ps = ps2.tile([128, 128], F32, name="psiT_ps", tag="p2")
                nc.tensor.transpose(
                    psiT_ps, psi[:, 128 * h:128 * (h + 1)], id_sb[:, :])
                psiT = work.tile([128, 128], F32, name="psiT", tag="psiT")
                nc.vector.tensor_copy(psiT, psiT_ps)
                phiT_ps = ps2.tile([128, 128], F32, name="phiT_ps", tag="p2")
                nc.tensor.matmul(phiT_ps, lhsT=mbd_sb[:, :], rhs=psiT,
                                 start=True, stop=True)
                phi2 = work.tile([128, 128], F32, name="phi2", tag="phi2")
                nc.scalar.activation(phi2, phiT_ps, AF.Square)
                o10_ps = ps2.tile([80, 128], F32, name="o10_ps", tag="p2")
                nc.tensor.matmul(o10_ps, lhsT=pbd_sb[:, :], rhs=phi2,
                                 start=True, stop=True)
                # bias-add lands directly in the transposed output staging
                # tile; stored once at the end (keeps 512B-contiguous DMA
                # descriptors and no copy<->transpose ring transitions)
                hh = 2 * cki + h
                nc.scalar.activation(out2_sb[:, 128 * hh:128 * (hh + 1)],
                                     o10_ps, AF.Identity, bias=pb80_sb[:, :])

        # single store at the very end, in transposed layout [64, 10, 128];
        # the host flips it back to [8192, 10]
        nc.scalar.dma_start(
            out[:, :, :].rearrange("(h t) c p -> (t c) h p", h=8),
            out2_sb[:, :].rearrange("p (h b) -> p h b", h=8))

    nc.finalize()  # bacc: register alloc + event-semaphore wait splitting
    return nc


_NC_CACHE: dict = {}


def _get_nc() -> bass.Bass:
    if "nc" not in _NC_CACHE:
        _NC_CACHE["nc"] = build_nc()
    return _NC_CACHE["nc"]


def make_in_maps(inputs: dict) -> list:
    x = np.asarray(inputs["input_features"], np.float32)
    pre_w = np.asarray(inputs["pre_w"], np.float32)
    pre_b = np.asarray(inputs["pre_b"], np.float32)
    q_params = np.asarray(inputs["q_params"], np.float32)
    post_w = np.asarray(inputs["post_w"], np.float32)
    post_b = np.asarray(inputs["post_b"], np.float32)

    M = _build_M(q_params)
    P = _build_P(post_w)
    mbd = np.zeros((128, 128), np.float32)
    pbd = np.zeros((128, 80), np.float32)
    for t in range(8):
        mbd[16 * t:16 * (t + 1), 16 * t:16 * (t + 1)] = M.T
        pbd[16 * t:16 * (t + 1), 10 * t:10 * (t + 1)] = P
    # pre_wt_sb[p, 4k+f] = pre_w[f, 128k+p]
    pre_wt = np.ascontiguousarray(
        pre_w.T.reshape(4, 128, 4).transpose(1, 0, 2).reshape(128, 16)
    ).astype(ml_dtypes.bfloat16)
    bias_pack = np.zeros((1, 132), dtype=ml_dtypes.bfloat16)
    bias_pack[0, :128] = 1.0
    bias_pack[0, 128:] = pre_b.astype(ml_dtypes.bfloat16)
    post_b80 = np.ascontiguousarray(np.tile(post_b, 8).reshape(80, 1))
    trigb = np.ascontiguousarray(np.broadcast_to(
        np.array([3.0 * PI4, PI4], np.float32), (128, 2)))
    ident = np.eye(128, dtype=np.float32)

    xb = x.astype(ml_dtypes.bfloat16)
    consts = dict(pre_wt=pre_wt, bias_pack=bias_pack, mbd=mbd, pbd=pbd,
                  post_b80=post_b80, trigb=trigb, ident=ident)
    return [dict(x=xb[B * i:B * (i + 1)], **consts) for i in range(N_CORES)]


def unpack_out(dev_out: np.ndarray) -> np.ndarray:
    """[TILES, C, 128] device layout -> [B, C]."""
    return dev_out.transpose(0, 2, 1).reshape(B, C)


def run_on_device(inputs: dict, **kwargs):
    """Returns (full_output, BassKernelResults)."""
    nc = _get_nc()
    in_maps = make_in_maps(inputs)
    res = run_bass_kernel_spmd(nc, in_maps, core_ids=list(range(N_CORES)),
                               **kwargs)
    full = np.concatenate(
        [unpack_out(res.results[i]["out"]) for i in range(N_CORES)], 0)
    return np.ascontiguousarray(full, dtype=np.float32), res


def kernel(**inputs) -> np.ndarray:
    out, _ = run_on_device(inputs)
    return out


# revision 47
# speedup vs baseline: 1.7456x; 1.0205x over previous
"""Trainium2 Bass kernel for nn_DressedQuantumNet.

Math reformulation (exact, up to float rounding):
  pre_out = x @ pre_w.T + pre_b                  # [B,4]
  theta_w = (pi/4)*tanh(pre_out_w) + pi/4        # in (0, pi/2)
  v_w     = [cos theta_w, sin theta_w]           # per-qubit state (positive)
  psi     = v_0 (x) v_1 (x) v_2 (x) v_3          # [B,16] product state
  phi     = M @ psi        # M = fixed 16x16 matrix of the CNOT/RY circuit
  out     = (phi*phi)^T P + post_b  # P[i,c] = sum_w post_w[c,w] * z_w(i)

Device strategy (pure data parallel over 8 cores, 8192 samples each):
  - x is downcast to bf16 on host (halves HBM traffic; fp32 accumulation in
    PSUM keeps the matmul accurate).
  - x tiles are loaded transposed via the DMA xbar (dma transpose), so the
    contraction dim (D=512, in 4 chunks of 128) lands on SBUF partitions.
  - pre-matmul: lhsT = xT chunk [128d, 128b], rhs = pre_w^T chunk [128d, 4]
    accumulated over the 4 chunks into PSUM [128b, 4].
  - bias + PSUM->SBUF handled by one vector add with a broadcast bias AP.
  - angles/trig on ScalarE (Tanh + 2x Sin with scale/bias folding cos).
  - psi built with 3 broadcast-AP vector multiplies.
  - quantum circuit: PE transpose of psi -> [16 comps x 8 tiles, 128 samples],
    then two block-diagonal matmuls (M and P) on the tensor engine.
"""

import os
import sys

for _p in ("/opt/trn_rl_repo",):
    if os.path.isdir(_p) and _p not in sys.path:
        sys.path.insert(0, _p)

import math
import numpy as np
import ml_dtypes
from contextlib import ExitStack

import concourse.bass as bass
import concourse.bacc as bacc
import concourse.mybir as mybir
from concourse.tile import TileContext, add_dep_helper
from concourse.bass_utils import run_bass_kernel_spmd

F32 = mybir.dt.float32
BF16 = mybir.dt.bfloat16
AF = mybir.ActivationFunctionType
PI4 = math.pi / 4.0

N_CORES = 8
B_FULL, D, C = 65536, 512, 10
B = B_FULL // N_CORES          # 8192 samples per core
N_QUBITS, Q_DEPTH = 4, 6
TILES = B // 128               # 64 sample tiles of 128
GROUPS = 16                    # phase-1 groups of 512 samples (4 tiles)
CHUNKS = 4                     # phase-2 chunks of 2048 samples (16 tiles)
G_PER_C = GROUPS // CHUNKS


# ---------------------------------------------------------------- host math
def _apply_1q(state, gate, wire):
    state = np.moveaxis(state, wire, 0)
    state = np.tensordot(gate, state, axes=((1,), (0,)))
    return np.moveaxis(state, 0, wire)


def _apply_cnot(state, ctrl, tgt):
    state = np.moveaxis(state, (ctrl, tgt), (0, 1))
    state = np.stack([state[0], state[1][::-1]], axis=0)
    return np.moveaxis(state, (0, 1), (ctrl, tgt))


def _ry(theta):
    c, s = np.cos(theta * 0.5), np.sin(theta * 0.5)
    return np.array([[c, -s], [s, c]])


def _build_M(q_params: np.ndarray) -> np.ndarray:
    """16x16 matrix of the fixed part of the circuit (after the per-sample
    RY layer): 6 repetitions of [CNOT(0,1), CNOT(2,3), CNOT(1,2), RY layer]."""
    qw = np.asarray(q_params, np.float64).reshape(Q_DEPTH, N_QUBITS)
    M = np.zeros((16, 16), np.float64)
    for i in range(16):
        state = np.zeros(16, np.float64)
        state[i] = 1.0
        state = state.reshape((2,) * N_QUBITS)
        for k in range(Q_DEPTH):
            for a in range(0, N_QUBITS - 1, 2):
                state = _apply_cnot(state, a, a + 1)
            for a in range(1, N_QUBITS - 1, 2):
                state = _apply_cnot(state, a, a + 1)
            for w in range(N_QUBITS):
                state = _apply_1q(state, _ry(qw[k, w]), w)
        M[:, i] = state.reshape(16)
    return M


def _build_P(post_w: np.ndarray) -> np.ndarray:
    """P[i, c] = sum_w post_w[c, w] * z_w(i), where z_w(i) flips sign with
    bit (3-w) of the state index i (axis 0 of the state = qubit 0)."""
    post_w = np.asarray(post_w, np.float64)
    i = np.arange(16)
    z = np.stack([1.0 - 2.0 * ((i >> (3 - w)) & 1) for w in range(N_QUBITS)], 1)
    return z @ post_w.T  # [16, 10]


# ---------------------------------------------------------------- bass build
def build_nc(sim_compat: bool = False) -> bass.Bass:
    # Bacc (not raw Bass): its finalize() runs generate_event_semaphores,
    # which splits multi-semaphore waits to satisfy the TRN2 one-wait-per-
    # instruction ISA limit.
    nc = bacc.Bacc(None)
    x = nc.dram_tensor("x", [B, D], BF16, kind="ExternalInput")
    pre_wt = nc.dram_tensor("pre_wt", [128, 16], BF16, kind="ExternalInput")
    pre_b41 = nc.dram_tensor("pre_b41", [4, 1], F32, kind="ExternalInput")
    mbd = nc.dram_tensor("mbd", [128, 128], F32, kind="ExternalInput")
    pbd = nc.dram_tensor("pbd", [128, 80], F32, kind="ExternalInput")
    post_b80 = nc.dram_tensor("post_b80", [80, 1], F32, kind="ExternalInput")
    trigb = nc.dram_tensor("trigb", [128, 2], F32, kind="ExternalInput")
    ident = nc.dram_tensor("ident", [128, 128], F32, kind="ExternalInput")
    # transposed on device: out[tile, class, sample-in-tile]; host flips back
    out = nc.dram_tensor("out", [TILES, C, 128], F32, kind="ExternalOutput")

    with ExitStack() as ctx:
        tc = ctx.enter_context(TileContext(nc))
        consts = ctx.enter_context(tc.tile_pool(name="consts", bufs=1))
        # all 64 xT tiles stay resident (8 MB) — avoids WAR waits on the
        # transpose DMAs (DmaTransposeAnt supports very few sync waits)
        xt_pool = ctx.enter_context(tc.tile_pool(name="xt", bufs=GROUPS))
        work = ctx.enter_context(tc.tile_pool(name="work", bufs=2))
        ps_po = ctx.enter_context(tc.tile_pool(name="ps_po", space="PSUM", bufs=2))
        ps2 = ctx.enter_context(tc.tile_pool(name="ps2", space="PSUM", bufs=4))

        pre_wt_sb = consts.tile([128, 16], BF16)
        nc.gpsimd.dma_start(pre_wt_sb, pre_wt[:, :])
        pre_b_sb = consts.tile([4, 1], F32)
        nc.gpsimd.dma_start(pre_b_sb, pre_b41[:, :])
        mbd_sb = consts.tile([128, 128], F32)
        nc.gpsimd.dma_start(mbd_sb, mbd[:, :])
        pbd_sb = consts.tile([128, 80], F32)
        nc.gpsimd.dma_start(pbd_sb, pbd[:, :])
        pb80_sb = consts.tile([80, 1], F32)
        nc.gpsimd.dma_start(pb80_sb, post_b80[:, :])
        trigb_sb = consts.tile([128, 2], F32)
        nc.gpsimd.dma_start(trigb_sb, trigb[:, :])
        id_sb = consts.tile([128, 128], F32)
        last_const = nc.gpsimd.dma_start(id_sb, ident[:, :])

        out2_sb = consts.tile([80, 128 * 2 * CHUNKS], F32)  # [80, 1024]
        # feature-row tanh staging, bf16, padded to 16 partitions so the
        # SBUF->SBUF xbar transpose (rows%16==0) can flip it to sample-major;
        # rows 4..15 are never written or consumed
        t16_sb = consts.tile([16, B], BF16)  # [16, 8192]
        nc.gpsimd.memset(t16_sb[:, :], 0.0)  # init the padding rows once

        # pin the activation table to silu_and_others once: it contains
        # silu+tanh+sin+square+identity, so no further table loads happen.
        # (CoreSim can't evaluate Silu; the sim build substitutes Tanh —
        # the value is unused either way.)
        silu_sb = consts.tile([128, 1], F32)
        nc.scalar.activation(silu_sb, trigb_sb[:, 0:1],
                             AF.Tanh if sim_compat else AF.Silu)

        for cki in range(CHUNKS):
            # ---- phase 1: pre-net for this chunk's 4 groups of 512 samples
            for gi in range(G_PER_C):
                g = cki * G_PER_C + gi
                # one xbar transpose per 512-sample group with a fully
                # contiguous 512KB DRAM source: out[p, k, b] = x[b, 128k+p];
                # alternate between the two HWDGE queues (SP / ACT)
                xt = xt_pool.tile([128, 4 * 512], BF16, name="xt", tag="xt")
                xpose = nc.sync.dma_start(
                    xt[:, :].rearrange("p (k b) -> p k b", k=4),
                    x[512 * g:512 * (g + 1), :],
                    transpose=True)
                # keep all plain copies scheduled before all xbar transposes
                # (every copy<->transpose transition serializes the DMA ring)
                add_dep_helper(last_const.ins, xpose.ins, sync=False,
                               reason="consts before xbar transposes")
                # pre-out transposed: lhsT is the tiny pre_w chunk (4-column
                # LDWEIGHTS), xT streams as the moving operand at 1 col/cycle
                po = ps_po.tile([4, 512], F32, name="po", tag="po")
                for k in range(4):
                    nc.tensor.matmul(
                        po[:, :],
                        lhsT=pre_wt_sb[:, 4 * k:4 * (k + 1)],
                        rhs=xt[:, 512 * k:512 * (k + 1)],
                        start=(k == 0), stop=(k == 3))
                # fused bias + tanh, straight out of PSUM, bf16 out
                nc.scalar.activation(t16_sb[0:4, 512 * g:512 * (g + 1)], po,
                                     AF.Tanh, bias=pre_b_sb[:, :])

            # ---- phase 2: trig + quantum net for this chunk (16 tiles)
            # back to sample-major layout with one tiny xbar transpose (same
            # DMA mode as the x loads, so no ring-mode transitions):
            # th_bf[p, t, i] = t16[i, 2048*cki + 128t + p]
            th_bf = work.tile([128, 256], BF16, name="th_bf", tag="th_bf")
            nc.sync.dma_start(
                th_bf[:, :].rearrange("p (t i) -> p t i", i=16),
                t16_sb[:, 2048 * cki:2048 * (cki + 1)],
                transpose=True)
            cs = work.tile([128, 128], F32, name="cs", tag="cs")
            cs4 = cs[:, :].rearrange("p (t w x) -> p t w x", w=4, x=2)
            th3 = th_bf[:, :].rearrange("p (t i) -> p t i", i=16)[:, :, 0:4]
            # cos(theta) = sin(pi/4*tanh + 3pi/4); sin(theta) = sin(.. + pi/4)
            nc.scalar.activation(cs4[:, :, :, 0], th3, AF.Sin,
                                 bias=trigb_sb[:, 0:1], scale=PI4)
            nc.scalar.activation(cs4[:, :, :, 1], th3, AF.Sin,
                                 bias=trigb_sb[:, 1:2], scale=PI4)
            cs8 = cs[:, :].rearrange("p (t w) -> p t w", w=8)
            v01 = work.tile([128, 64], F32, name="v01", tag="v01")
            v23 = work.tile([128, 64], F32, name="v23", tag="v23")
            nc.vector.tensor_tensor(
                out=v01[:, :].rearrange("p (t a b) -> p t a b", a=2, b=2),
                in0=cs8[:, :, 0:2].unsqueeze(3).broadcast_to((128, 16, 2, 2)),
                in1=cs8[:, :, 2:4].unsqueeze(2).broadcast_to((128, 16, 2, 2)),
                op=mybir.AluOpType.mult)
            nc.vector.tensor_tensor(
                out=v23[:, :].rearrange("p (t a b) -> p t a b", a=2, b=2),
                in0=cs8[:, :, 4:6].unsqueeze(3).broadcast_to((128, 16, 2, 2)),
                in1=cs8[:, :, 6:8].unsqueeze(2).broadcast_to((128, 16, 2, 2)),
                op=mybir.AluOpType.mult)
            psi = work.tile([128, 256], F32, name="psi", tag="psi")
            nc.vector.tensor_tensor(
                out=psi[:, :].rearrange("p (t a b) -> p t a b", a=4, b=4),
                in0=v01[:, :].rearrange("p (t i) -> p t i", i=4)
                    .unsqueeze(3).broadcast_to((128, 16, 4, 4)),
                in1=v23[:, :].rearrange("p (t i) -> p t i", i=4)
                    .unsqueeze(2).broadcast_to((128, 16, 4, 4)),
                op=mybir.AluOpType.mult)

            for h in range(2):
                psiT_ps = ps2.tile([128, 128], F32, name="psiT_ps", tag="p2")
                nc.tensor.transpose(
                    psiT_ps, psi[:, 128 * h:128 * (h + 1)], id_sb[:, :])
                psiT = work.tile([128, 128], F32, name="psiT", tag="psiT")
                nc.vector.tensor_copy(psiT, psiT_ps)
                phiT_ps = ps2.tile([128, 128], F32, name="phiT_ps", tag="p2")
                nc.tensor.matmul(phiT_ps, lhsT=mbd_sb[:, :], rhs=psiT,
                                 start=True, stop=True)
                phi2 = work.tile([128, 128], F32, name="phi2", tag="phi2")
                nc.scalar.activation(phi2, phiT_ps, AF.Square)
                o10_ps = ps2.tile([80, 128], F32, name="o10_ps", tag="p2")
                nc.tensor.matmul(o10_ps, lhsT=pbd_sb[:, :], rhs=phi2,
                                 start=True, stop=True)
                # bias-add lands directly in the transposed output staging
                # tile; stored once at the end (keeps 512B-contiguous DMA
                # descriptors and no copy<->transpose ring transitions)
                hh = 2 * cki + h
                nc.scalar.activation(out2_sb[:, 128 * hh:128 * (hh + 1)],
                                     o10_ps, AF.Identity, bias=pb80_sb[:, :])

        # single store at the very end, in transposed layout [64, 10, 128];
        # the host flips it back to [8192, 10]
        nc.scalar.dma_start(
            out[:, :, :].rearrange("(h t) c p -> (t c) h p", h=8),
            out2_sb[:, :].rearrange("p (h b) -> p h b", h=8))

    nc.finalize()  # bacc: register alloc + event-semaphore wait splitting
    return nc


_NC_CACHE: dict = {}


def _get_nc() -> bass.Bass:
    if "nc" not in _NC_CACHE:
        _NC_CACHE["nc"] = build_nc()
    return _NC_CACHE["nc"]


def make_in_maps(inputs: dict) -> list:
    x = np.asarray(inputs["input_features"], np.float32)
    pre_w = np.asarray(inputs["pre_w"], np.float32)
    pre_b = np.asarray(inputs["pre_b"], np.float32)
    q_params = np.asarray(inputs["q_params"], np.float32)
    post_w = np.asarray(inputs["post_w"], np.float32)
    post_b = np.asarray(inputs["post_b"], np.float32)

    M = _build_M(q_params)
    P = _build_P(post_w)
    mbd = np.zeros((128, 128), np.float32)
    pbd = np.zeros((128, 80), np.float32)
    for t in range(8):
        mbd[16 * t:16 * (t + 1), 16 * t:16 * (t + 1)] = M.T
        pbd[16 * t:16 * (t + 1), 10 * t:10 * (t + 1)] = P
    # pre_wt_sb[p, 4k+f] = pre_w[f, 128k+p]
    pre_wt = np.ascontiguousarray(
        pre_w.T.reshape(4, 128, 4).transpose(1, 0, 2).reshape(128, 16)
    ).astype(ml_dtypes.bfloat16)
    pre_b41 = np.ascontiguousarray(pre_b.reshape(4, 1), dtype=np.float32)
    post_b80 = np.ascontiguousarray(np.tile(post_b, 8).reshape(80, 1))
    trigb = np.ascontiguousarray(np.broadcast_to(
        np.array([3.0 * PI4, PI4], np.float32), (128, 2)))
    ident = np.eye(128, dtype=np.float32)

    xb = x.astype(ml_dtypes.bfloat16)
    consts = dict(pre_wt=pre_wt, pre_b41=pre_b41, mbd=mbd, pbd=pbd,
                  post_b80=post_b80, trigb=trigb, ident=ident)
    return [dict(x=xb[B * i:B * (i + 1)], **consts) for i in range(N_CORES)]


def unpack_out(dev_out: np.ndarray) -> np.ndarray:
    """[TILES, C, 128] device layout -> [B, C]."""
    return dev_out.transpose(0, 2, 1).reshape(B, C)


def run_on_device(inputs: dict, **kwargs):
    """Returns (full_output, BassKernelResults)."""
    nc = _get_nc()
    in_maps = make_in_maps(inputs)
    res = run_bass_kernel_spmd(nc, in_maps, core_ids=list(range(N_CORES)),
                               **kwargs)
    full = np.concatenate(
        [unpack_out(res.results[i]["out"]) for i in range(N_CORES)], 0)
    return np.ascontiguousarray(full, dtype=np.float32), res


def kernel(**inputs) -> np.ndarray:
    out, _ = run_on_device(inputs)
    return out
